# revision 14
# baseline (speedup 1.0000x reference)
"""BERT layer (B=8, S=1024, E=1024, F=4096) on 8 trn2 NeuronCores.

Strategy: pure data-parallel over batch (1 element per core, no collectives).
Per-core kernel keeps activations feature-major ([features, tokens]) so every
weight matmul uses the natural [in, out] weight block as the PE stationary
operand. All matmul operands are fp16 (1 cy/col on the PE like fp32r, but
half the DMA, half the LDWEIGHTS time, and 2x DVE throughput for the
LayerNorm chains); PSUM accumulation and LN statistics stay fp32.
LayerNorm stats (reductions over the feature/partition dim) run on the PE
via ones-vector matmuls, interleaved into the producing loops; per-token
stats are broadcast across partitions with a K=1 ones-row matmul into PSUM,
then cast to fp16 for the 2x-rate normalize. Reciprocals use the
single-instruction DVE approx (~18 bits).
"""

import sys

for _p in ("/opt/trn_rl_repo", "/root/.axon_site/_ro/trn_rl_repo"):
    if _p not in sys.path:
        sys.path.append(_p)

import numpy as np

import concourse.bass as bass  # noqa: F401
import concourse.mybir as mybir
from concourse import bacc
from concourse.bass_utils import run_bass_kernel_spmd  # noqa: F401
from concourse.tile import TileContext

B, S, E, F = 8, 1024, 1024, 4096
P = 128
NE = E // P     # 8 tiles along E
NF = F // P     # 32 tiles along F
NS = S // P     # 8 tiles along S
C = 512         # free-dim chunk (one fp32 psum bank)
NC = S // C     # 2 chunks along S
EPS = 1e-12
AF = mybir.ActivationFunctionType
ALU = mybir.AluOpType
F32 = mybir.dt.float32
BF16 = mybir.dt.float16
NPBF16 = mybir.dt.np(BF16)


def _ln_scalars(nc, SM, psum_pool, s1, s2, dim, epst, ones_row, pfx):
    """From column-sum psums s1=sum(z), s2=sum(z^2) [1,C] build two fp16
    SBUF broadcast tiles [P,C]: bcax = rstd, bcbx = -mu*rstd."""
    # 3 rotating 2KB scratch slots (tag ring, bufs=1 => same address + WAR dep)
    negmu = SM.tile([1, C], F32, tag="lnt0", name=f"negmu_{pfx}")
    musq = SM.tile([1, C], F32, tag="lnt1", name=f"musq_{pfx}")
    var = SM.tile([1, C], F32, tag="lnt2", name=f"var_{pfx}")
    std = SM.tile([1, C], F32, tag="lnt1", name=f"std_{pfx}")
    rstd = SM.tile([1, C], F32, tag="lnt2", name=f"rstd_{pfx}")
    nmr = SM.tile([1, C], BF16, tag="lnt3", name=f"nmr_{pfx}")
    rstd_r = SM.tile([1, C], BF16, tag="lnt4", name=f"rstdr_{pfx}")
    # negmu = -s1/dim ; var = s2/dim - mu^2 ; std = sqrt(var + eps)
    # (each DVE op reads at most one PSUM operand)
    nc.vector.tensor_scalar_mul(negmu[:], s1[:], -1.0 / dim)
    nc.vector.tensor_tensor(musq[:], negmu[:], negmu[:], op=ALU.mult)
    nc.vector.scalar_tensor_tensor(
        var[:], s2[:], 1.0 / dim, musq[:], op0=ALU.mult, op1=ALU.subtract
    )
    nc.scalar.activation(std[:], var[:], AF.Sqrt, bias=epst[0:1, 0:1])
    nc.vector.reciprocal_approx_fast(out=rstd[:], in_=std[:])
    nc.vector.tensor_tensor(nmr[:], negmu[:], rstd[:], op=ALU.mult)
    nc.vector.tensor_copy(rstd_r[:], rstd[:])
    bca = psum_pool.tile([P, C], F32, tag="bca", bufs=1, name=f"bca_{pfx}")
    bcb = psum_pool.tile([P, C], F32, tag="bcb", bufs=1, name=f"bcb_{pfx}")
    nc.tensor.matmul(bca[:], ones_row[:], rstd_r[:], start=True, stop=True)
    nc.tensor.matmul(bcb[:], ones_row[:], nmr[:], start=True, stop=True)
    bcax = SM.tile([P, C], BF16, tag="bcax", name=f"bcax_{pfx}")
    bcbx = SM.tile([P, C], BF16, tag="bcbx", name=f"bcbx_{pfx}")
    nc.vector.tensor_copy(bcax[:], bca[:])
    nc.vector.tensor_copy(bcbx[:], bcb[:])
    return bcax, bcbx


def _ln_normalize(nc, R3, z_tile, bcax, bcbx, g, b, n, dst_ap):
    """dst = ((z * rstd) + (-mu*rstd)) * g[n] + b[n] for one [P, C] tile.
    All-fp16 tensor ops -> 2x DVE rate."""
    t1 = R3.tile([P, C], BF16, tag="tmp")
    nc.vector.tensor_mul(t1[:], z_tile, bcax[:])
    nc.vector.tensor_add(t1[:], t1[:], bcbx[:])
    nc.scalar.activation(
        dst_ap, t1[:], AF.Identity, bias=b[:, n:n + 1], scale=g[:, n:n + 1]
    )


def build():
    nc = bacc.Bacc("TRN2", target_bir_lowering=False, debug=False)

    xT_d = nc.dram_tensor("xT", [E, S], BF16, kind="ExternalInput")
    wq_d = nc.dram_tensor("wq", [NE, NE, P, P], BF16, kind="ExternalInput")
    wk_d = nc.dram_tensor("wk", [NE, NE, P, P], BF16, kind="ExternalInput")
    wv_d = nc.dram_tensor("wv", [E, E], BF16, kind="ExternalInput")
    wd_d = nc.dram_tensor("wd", [NE, NE, P, P], BF16, kind="ExternalInput")
    wi_d = nc.dram_tensor("wi", [NF, NE, P, P], BF16, kind="ExternalInput")
    wo_d = nc.dram_tensor("wo", [NE, 4, 8, P, P], BF16, kind="ExternalInput")
    # bias columns: [bq/32, bk, bv, bd, g1, b1, bo, g2, b2] -> [P, 9*NE]
    bias_d = nc.dram_tensor("biases", [P, 9 * NE], F32, kind="ExternalInput")
    bi_d = nc.dram_tensor("bi_cols", [P, NF], F32, kind="ExternalInput")
    ones_d = nc.dram_tensor("ones_in", [P, 1], BF16, kind="ExternalInput")
    onesrow_d = nc.dram_tensor("onesrow_in", [1, P], BF16, kind="ExternalInput")
    outT_d = nc.dram_tensor("outT", [E, S], F32, kind="ExternalOutput")

    with TileContext(nc) as tc:
        with (
            tc.tile_pool(name="persist", bufs=1) as PP,
            tc.tile_pool(name="wstage", bufs=4) as WS,
            tc.tile_pool(name="small", bufs=1) as SM,
            tc.tile_pool(name="rot3", bufs=3) as R3,
            tc.tile_pool(name="rot2", bufs=2) as R2,
        ):
            # ---- constants ----
            ones = SM.tile([P, 1], BF16, tag="ones")
            nc.sync.dma_start(ones[:], ones_d[:])
            ones_row = SM.tile([1, P], BF16, tag="onesrow")
            nc.sync.dma_start(ones_row[:], onesrow_d[:])
            epst = SM.tile([1, 1], F32, tag="epst")
            nc.vector.memset(epst[:], EPS)
            biases = SM.tile([P, 9 * NE], F32, tag="biases")
            nc.sync.dma_start(biases[:], bias_d[:])
            bq = biases[:, 0 * NE:1 * NE]   # bq/32
            bk = biases[:, 1 * NE:2 * NE]
            bd = biases[:, 3 * NE:4 * NE]
            g1 = biases[:, 4 * NE:5 * NE]
            b1 = biases[:, 5 * NE:6 * NE]
            bo = biases[:, 6 * NE:7 * NE]
            g2 = biases[:, 7 * NE:8 * NE]
            b2 = biases[:, 8 * NE:9 * NE]
            bicol = SM.tile([P, NF], F32, tag="bicol")
            nc.sync.dma_start(bicol[:], bi_d[:])

            # ---- xT loaded tile-by-tile inside the v-phase loop ----
            xT = PP.tile([P, NE, S], BF16, tag="xT")

            # ================= v = x @ Wv (token-major, no bias) ============
            v_sb = PP.tile([P, NS, E], BF16, tag="v")
            with tc.tile_pool(name="pv", bufs=1, space="PSUM") as PV:
                for c in range(NC):
                    pvs = [
                        PV.tile([P, C], F32, tag=f"pv{s_t}", name=f"pv{s_t}_{c}")
                        for s_t in range(NS)
                    ]
                    for k in range(NE):
                        if c == 0:
                            nc.sync.dma_start(
                                xT[:, k, :], xT_d[k * P:(k + 1) * P, :]
                            )
                        wvst = WS.tile([P, C], BF16, tag="wvst", bufs=3)
                        nc.sync.dma_start(
                            wvst[:], wv_d[k * P:(k + 1) * P, c * C:(c + 1) * C]
                        )
                        for s_t in range(NS):
                            nc.tensor.matmul(
                                pvs[s_t][:],
                                xT[:, k, s_t * P:(s_t + 1) * P],
                                wvst[:],
                                start=(k == 0),
                                stop=(k == NE - 1),
                            )
                    for s_t in range(NS):
                        nc.vector.tensor_copy(
                            v_sb[:, s_t, c * C:(c + 1) * C], pvs[s_t][:]
                        )

            # ================= qT / kT ======================================
            qT = PP.tile([P, NE, S], BF16, tag="qT")
            kT = PP.tile([P, NE, S], BF16, tag="kT")
            with tc.tile_pool(name="pqk", bufs=3, space="PSUM") as PQK:
                for (w_d, dst, bias_ap, scale) in (
                    (wq_d, qT, bq, 1.0 / 32.0),
                    (wk_d, kT, bk, 1.0),
                ):
                    for n in range(NE):
                        wst = WS.tile([P, NE, P], BF16, tag="wst")
                        nc.sync.dma_start(
                            wst[:], w_d[n].rearrange("k p m -> p k m")
                        )
                        for c in range(NC):
                            ps = PQK.tile([P, C], F32, tag="pqk")
                            for k in range(NE):
                                nc.tensor.matmul(
                                    ps[:],
                                    wst[:, k, :],
                                    xT[:, k, c * C:(c + 1) * C],
                                    start=(k == 0),
                                    stop=(k == NE - 1),
                                )
                            nc.scalar.activation(
                                dst[:, n, c * C:(c + 1) * C], ps[:],
                                AF.Identity,
                                bias=bias_ap[:, n:n + 1], scale=scale,
                            )

            # ================= attention ====================================
            # Order: sc0, av0, sc1, av1 -- the score/denominator pools (A) and
            # the av pool (B) both close before the post-attention pool opens,
            # keeping PSUM within 8 banks.
            z1 = PP.tile([P, NE, S], BF16, tag="kT")  # reuses kT slot
            h1 = PP.tile([P, NE, S], BF16, tag="qT")  # reuses qT slot

            ATT_B = tc.tile_pool(name="attB", bufs=1, space="PSUM")
            PB = ATT_B.__enter__()
            ATT_A = tc.tile_pool(name="attA", bufs=1, space="PSUM")
            PA = ATT_A.__enter__()

            def attn_block(qb):
                qs = slice(qb * C, (qb + 1) * C)
                wT = PP.tile([P, NS, C], BF16, tag="wT", name=f"wT{qb}")
                for j in range(NS):
                    ps_sT = PA.tile([P, C], F32, tag="psc", bufs=2, name=f"ps_sT{qb}_{j}")
                    for k in range(NE):
                        nc.tensor.matmul(
                            ps_sT[:],
                            kT[:, k, j * P:(j + 1) * P],
                            qT[:, k, qs],
                            start=(k == 0),
                            stop=(k == NE - 1),
                        )
                    nc.scalar.activation(wT[:, j, :], ps_sT[:], AF.Exp)
                ps_den = PA.tile([1, C], F32, tag="pden", bufs=1, name=f"psden{qb}")
                for j in range(NS):
                    nc.tensor.matmul(
                        ps_den[:], ones[:], wT[:, j, :],
                        start=(j == 0), stop=(j == NS - 1),
                    )
                rec = SM.tile([1, C], F32, tag="rcp", name=f"rec{qb}")
                nc.vector.reciprocal_approx_fast(out=rec[:], in_=ps_den[:])
                rec_r = SM.tile([1, C], BF16, tag="rcpr", name=f"recr{qb}")
                nc.vector.tensor_copy(rec_r[:], rec[:])
                bcq = PA.tile([P, C], F32, tag="bcq", bufs=1, name=f"bcq{qb}")
                nc.tensor.matmul(
                    bcq[:], ones_row[:], rec_r[:], start=True, stop=True,
                )
                recb = SM.tile([P, C], F32, tag="rcb", name=f"recb{qb}")
                nc.vector.tensor_copy(recb[:], bcq[:])
                return wT, recb

            def attn_av(qb, wT, recb):
                astg = PP.tile([P, NE, C], BF16, tag="astg", bufs=2, name=f"astg{qb}")
                for e_t in range(NE):
                    ps_a = PB.tile([P, C], F32, tag="pav", bufs=2, name=f"ps_a{qb}_{e_t}")
                    for j in range(NS):
                        nc.tensor.matmul(
                            ps_a[:],
                            v_sb[:, j, e_t * P:(e_t + 1) * P],
                            wT[:, j, :],
                            start=(j == 0),
                            stop=(j == NS - 1),
                        )
                    nc.vector.tensor_mul(astg[:, e_t, :], ps_a[:], recb[:])
                return astg

            wT0, recb0 = attn_block(0)
            astg0 = attn_av(0, wT0, recb0)
            wT1, recb1 = attn_block(1)
            ATT_A.__exit__(None, None, None)
            astg1 = attn_av(1, wT1, recb1)
            ATT_B.__exit__(None, None, None)

            # ====== post-attention scope: Wd + LN1 + FF + LN2 ==============
            # PSUM tags: pwd(2) + pstat1 + pstat2 + bca + bcb + pmm(2) = 8
            with tc.tile_pool(name="postpsum", bufs=1, space="PSUM") as PO:

                def wd_chunk(c, astg):
                    """Wd matmuls + z1 evac + inline LN1 stats for chunk c."""
                    cs = slice(c * C, (c + 1) * C)
                    s1 = PO.tile([1, C], F32, tag="pstat1", bufs=1, name=f"l1s1_{c}")
                    s2 = PO.tile([1, C], F32, tag="pstat2", bufs=1, name=f"l1s2_{c}")
                    for n in range(NE):
                        wst = WS.tile([P, NE, P], BF16, tag="wst", name=f"wdst{c}_{n}")
                        nc.sync.dma_start(
                            wst[:], wd_d[n].rearrange("k p m -> p k m")
                        )
                        ps = PO.tile([P, C], F32, tag="pwd", bufs=2, name=f"pwd{c}_{n}")
                        for k in range(NE):
                            nc.tensor.matmul(
                                ps[:],
                                wst[:, k, :],
                                astg[:, k, :],
                                start=(k == 0),
                                stop=(k == NE - 1),
                            )
                        nc.vector.scalar_tensor_tensor(
                            z1[:, n, cs],
                            ps[:], bd[:, n:n + 1],
                            xT[:, n, cs],
                            op0=ALU.add, op1=ALU.add,
                        )
                        zsq = R2.tile([P, C], BF16, tag="zsq", bufs=1, name=f"zsq1_{c}_{n}")
                        nc.scalar.activation(zsq[:], z1[:, n, cs], AF.Square)
                        nc.tensor.matmul(
                            s1[:], ones[:], z1[:, n, cs],
                            start=(n == 0), stop=(n == NE - 1),
                        )
                        nc.tensor.matmul(
                            s2[:], ones[:], zsq[:],
                            start=(n == 0), stop=(n == NE - 1),
                        )
                    return s1, s2

                # Wd(c0) -> LN1 scalars(c0) -> Wd(c1) -> norm(c0) -> ...
                s1_0, s2_0 = wd_chunk(0, astg0)
                bcax0, bcbx0 = _ln_scalars(
                    nc, SM, PO, s1_0, s2_0, E, epst, ones_row, "l1c0"
                )
                s1_1, s2_1 = wd_chunk(1, astg1)
                for n in range(NE):
                    _ln_normalize(
                        nc, R3, z1[:, n, 0:C], bcax0, bcbx0, g1, b1, n,
                        h1[:, n, 0:C],
                    )
                bcax1, bcbx1 = _ln_scalars(
                    nc, SM, PO, s1_1, s2_1, E, epst, ones_row, "l1c1"
                )
                for n in range(NE):
                    _ln_normalize(
                        nc, R3, z1[:, n, C:2 * C], bcax1, bcbx1, g1, b1, n,
                        h1[:, n, C:2 * C],
                    )

                # ================= FF =======================================
                for c in range(NC):
                    cs = slice(c * C, (c + 1) * C)
                    ffA = PP.tile([P, NF // 2, C], BF16, tag="xT")
                    ffB = PP.tile([P, NF // 2, C], BF16, tag="v")
                    for f in range(NF):
                        wst = WS.tile([P, NE, P], BF16, tag="wst")
                        nc.sync.dma_start(
                            wst[:], wi_d[f].rearrange("k p m -> p k m")
                        )
                        ps = PO.tile([P, C], F32, tag="pmm", bufs=2)
                        for k in range(NE):
                            nc.tensor.matmul(
                                ps[:],
                                wst[:, k, :],
                                h1[:, k, cs],
                                start=(k == 0),
                                stop=(k == NE - 1),
                            )
                        dst = ffA if f < NF // 2 else ffB
                        nc.scalar.activation(
                            dst[:, f % (NF // 2), :], ps[:],
                            AF.Gelu, bias=bicol[:, f:f + 1],
                        )
                    # FF2 + bo + residual(h1) -> z2 chunk, LN2 stats inline
                    z2 = PP.tile([P, NE, C], BF16, tag="wT")
                    fs1 = PO.tile([1, C], F32, tag="pstat1", bufs=1, name=f"fs1_{c}")
                    fs2 = PO.tile([1, C], F32, tag="pstat2", bufs=1, name=f"fs2_{c}")
                    for n in range(NE):
                        pso = PO.tile([P, C], F32, tag="pmm", bufs=2)
                        for g in range(4):
                            wst = WS.tile([P, NE, P], BF16, tag="wst")
                            nc.sync.dma_start(
                                wst[:], wo_d[n, g].rearrange("k p m -> p k m")
                            )
                            for j in range(8):
                                f = g * 8 + j
                                src = ffA if f < NF // 2 else ffB
                                nc.tensor.matmul(
                                    pso[:],
                                    wst[:, j, :],
                                    src[:, f % (NF // 2), :],
                                    start=(f == 0),
                                    stop=(f == NF - 1),
                                )
                        nc.vector.scalar_tensor_tensor(
                            z2[:, n, :], pso[:], bo[:, n:n + 1],
                            h1[:, n, cs],
                            op0=ALU.add, op1=ALU.add,
                        )
                        zsq = R2.tile([P, C], BF16, tag="zsq", bufs=1, name=f"zsq2_{c}_{n}")
                        nc.scalar.activation(zsq[:], z2[:, n, :], AF.Square)
                        nc.tensor.matmul(
                            fs1[:], ones[:], z2[:, n, :],
                            start=(n == 0), stop=(n == NE - 1),
                        )
                        nc.tensor.matmul(
                            fs2[:], ones[:], zsq[:],
                            start=(n == 0), stop=(n == NE - 1),
                        )
                    # LN2 -> out
                    bcax, bcbx = _ln_scalars(
                        nc, SM, PO, fs1, fs2, E, epst, ones_row, f"l2c{c}"
                    )
                    for n in range(NE):
                        oe = R2.tile([P, C], F32, tag="outevac")
                        t1 = R3.tile([P, C], BF16, tag="tmp")
                        nc.vector.tensor_mul(t1[:], z2[:, n, :], bcax[:])
                        nc.vector.tensor_add(t1[:], t1[:], bcbx[:])
                        nc.scalar.activation(
                            oe[:], t1[:], AF.Identity,
                            bias=b2[:, n:n + 1], scale=g2[:, n:n + 1],
                        )
                        nc.sync.dma_start(
                            outT_d[n * P:(n + 1) * P, c * C:(c + 1) * C], oe[:]
                        )
    nc.compile()
    return nc


_RUNNER_CACHE = None


def _get_runner():
    """Compile once; return f(in_maps) -> list[dict] using a cached jitted
    shard_map executable (8 cores, no donation so device buffers reuse)."""
    global _RUNNER_CACHE
    if _RUNNER_CACHE is not None:
        return _RUNNER_CACHE

    import jax
    from jax.sharding import Mesh, PartitionSpec
    from jax.experimental.shard_map import shard_map
    from concourse import bass2jax

    nc = build()
    bass2jax.install_neuronx_cc_hook()

    partition_name = (
        nc.partition_id_tensor.name if nc.partition_id_tensor else None
    )
    in_names, out_names, out_avals = [], [], []
    for alloc in nc.m.functions[0].allocations:
        if not isinstance(alloc, mybir.MemoryLocationSet):
            continue
        name = alloc.memorylocations[0].name
        if alloc.kind == "ExternalInput":
            if name != partition_name:
                in_names.append(name)
        elif alloc.kind == "ExternalOutput":
            out_names.append(name)
            out_avals.append(
                jax.core.ShapedArray(
                    tuple(alloc.tensor_shape), mybir.dt.np(alloc.dtype)
                )
            )
    n_params = len(in_names)
    all_in_names = in_names + out_names
    if partition_name is not None:
        all_in_names = all_in_names + [partition_name]

    def _body(*args):
        operands = list(args)
        if partition_name is not None:
            operands.append(bass2jax.partition_id_tensor())
        outs = bass2jax._bass_exec_p.bind(
            *operands,
            out_avals=tuple(out_avals),
            in_names=tuple(all_in_names),
            out_names=tuple(out_names),
            lowering_input_output_aliases=(),
            sim_require_finite=True,
            sim_require_nnan=True,
            nc=nc,
        )
        return tuple(outs)

    devices = jax.devices()[:B]
    mesh = Mesh(np.asarray(devices), ("core",))
    n_all = n_params + len(out_names)
    sharded = jax.jit(
        shard_map(
            _body,
            mesh=mesh,
            in_specs=(PartitionSpec("core"),) * n_all,
            out_specs=(PartitionSpec("core"),) * len(out_names),
            check_rep=False,
        ),
        keep_unused=True,
    )

    def run(in_maps, device_args=None, timing_reps=0):
        if device_args is None:
            concat_in = [
                np.concatenate([np.asarray(m[nm]) for m in in_maps], axis=0)
                for nm in in_names
            ]
            concat_zeros = [
                np.zeros((B * a.shape[0], *a.shape[1:]), a.dtype) for a in out_avals
            ]
            device_args = [jax.device_put(a) for a in concat_in + concat_zeros]
        out_arrs = sharded(*device_args)
        jax.block_until_ready(out_arrs)
        timings = []
        for _ in range(timing_reps):
            import time as _time

            t0 = _time.perf_counter()
            out_arrs = sharded(*device_args)
            jax.block_until_ready(out_arrs)
            timings.append(_time.perf_counter() - t0)
        results = [
            {
                nm: np.asarray(out_arrs[i]).reshape(B, *out_avals[i].shape)[c]
                for i, nm in enumerate(out_names)
            }
            for c in range(B)
        ]
        return results, device_args, timings

    _RUNNER_CACHE = run
    return run


def _pretile(w, nt, kt):
    """W [K, N] -> [nt, kt, 128, 128] with tile[n][k] = W[kblk, nblk]."""
    t = w.reshape(kt, P, nt, P).transpose(2, 0, 1, 3)
    return np.ascontiguousarray(t)


def _cols(vec):
    """[X*128] -> [128, X] with col j = vec[j*128:(j+1)*128]."""
    return np.ascontiguousarray(vec.reshape(-1, P).T)


def _build_in_maps(inputs):
    inp = {k: np.asarray(v, dtype=np.float32) for k, v in inputs.items()}
    x = inp["hidden_states"]  # [B, S, E]

    wq = _pretile(inp["Wq"], NE, NE).astype(NPBF16)
    wk = _pretile(inp["Wk"], NE, NE).astype(NPBF16)
    wd = _pretile(inp["Wd"], NE, NE).astype(NPBF16)
    wi = _pretile(inp["Wi"], NF, NE).astype(NPBF16)
    wo = _pretile(inp["Wo"], NE, NF).reshape(NE, 4, 8, P, P).astype(NPBF16)
    wv = np.ascontiguousarray(inp["Wv"]).astype(NPBF16)

    bias_full = np.concatenate(
        [
            _cols(inp["bq"] / 32.0), _cols(inp["bk"]),
            _cols(np.zeros_like(inp["bv"])),
            _cols(inp["bd"] + inp["bv"] @ inp["Wd"]),
            _cols(inp["g1"]), _cols(inp["b1"]),
            _cols(inp["bo"]), _cols(inp["g2"]), _cols(inp["b2"]),
        ],
        axis=1,
    )
    bicol = _cols(inp["bi"])

    in_maps = []
    for bidx in range(B):
        xT = np.ascontiguousarray(x[bidx].T).astype(NPBF16)  # [E, S]
        in_maps.append(
            {
                "xT": xT, "wq": wq, "wk": wk, "wv": wv, "wd": wd,
                "wi": wi, "wo": wo, "biases": bias_full, "bi_cols": bicol,
                "ones_in": np.ones((P, 1), dtype=NPBF16),
                "onesrow_in": np.ones((1, P), dtype=NPBF16),
            }
        )
    return in_maps


def kernel(**inputs):
    run = _get_runner()
    results, _, _ = run(_build_in_maps(inputs))
    out = np.stack([r["outT"].T for r in results]).astype(np.float32)
    return out
